# revision 27
# baseline (speedup 1.0000x reference)
"""Trainium2 Bass kernel for CAttention (contextual attention).

Math (per batch element, derived from the reference):
    x:    (c=128, h=64, w=64), flat (128, 4096); m: (1, 4096)
    k    = normalize_rows(x.reshape(c, hw).T + eps)          # (4096, 128)
    y    = 3x3 zero-padded box filter of x                   # (128, 4096)
    S    = k @ y                                             # (4096 l, 4096 ij)
    att  = softmax over l (per column); u = exp(S - 20) (S bounded, col max
           >= ~11, so a constant shift suffices; att = u / colsum(u))
    rec  = k.T @ att                                         # (128, 4096)
    out  = rec * (1-m)/9 + x*m

Sharding: pure data parallel over batch (4) x output-column halves (2) = 8
cores, zero cross-core communication.

Engine split per core (all matmul operands bf16; 1 cyc/row, same rate as
f32r but half the SBUF/DMA, and 2x faster DVE adds):
  PE   : mm1 S-tiles, mm2 rec accumulation, ones-matmuls for column sums
         of tree-reduced u tiles.                              ~61us
  ACT  : 22 exp tiles per block (exact Exp, bf16 out, per-partition
         scale=rsqrt(norm), bias=-20) + first half of the norm squares
         (AF.Square with accum) during the DMA head.           ~64us
  DVE  : 10 exp tiles per block via the Schraudolph bit-trick (bf16 bits
         of exp(t) = int16(t*128/ln2 + 16256 - C), a single tensor_scalar
         f32->int16), y box filter (bf16 2x adds), second half of squares
         (tensor_tensor_reduce), rsqrt Newton, 4/7 of the column-sum tree,
         epilogue.                                             ~63us
  GPSIMD (cannot read PSUM -> no exp, no scalar_tensor_tensor): all of
         kn = xt*rs, 3/7 of the column-sum tree adds, blend prep. ~55us
Validated numerics: rel err ~2e-4 vs fp64 reference (gate 2e-2).
"""

import numpy as np
import ml_dtypes

NPBF16 = ml_dtypes.bfloat16

EPS = 1e-7
SHIFT = 20.0
LN2_INV_128 = 128.0 / float(np.log(2.0))   # 184.6617
SCHR_C = 5.5
SCHR_OFF = 16256.0 - SHIFT * LN2_INV_128 - SCHR_C
C = 128          # channels
L = 4096         # spatial locations (l axis)
HALF = 2048      # output columns per core
BLK = 1024       # ij block (psum-bank sized: 2 banks)
NLT = 32         # l tiles of 128
YW = 2176        # xyh width: 34 padded image rows x 64

# exp tiles per block handled by DVE (Schraudolph); the rest go to ACT
DVE_TILES = {4, 9, 14, 19, 24, 29}

_CACHE = {}


def _build_program():
    import concourse.bass as bass
    import concourse.bacc as bacc
    import concourse.tile as tile
    import concourse.mybir as mybir

    F32 = mybir.dt.float32
    BF = mybir.dt.bfloat16
    I16 = mybir.dt.int16
    I32 = mybir.dt.int32
    AF = mybir.ActivationFunctionType
    ALU = mybir.AluOpType

    nc = bacc.Bacc("TRN2", target_bir_lowering=False, num_swdge_queues=4)

    xb_d = nc.dram_tensor("xb", [C, L], BF, kind="ExternalInput")
    # xt pre-tiled on host to SBUF layout: xt[p, t*128+c] = x[c, t*128+p]
    xt_d = nc.dram_tensor("xt", [C, L], BF, kind="ExternalInput")
    xyh_d = nc.dram_tensor("xyh", [C, YW], BF, kind="ExternalInput")
    mrep_d = nc.dram_tensor("mrep", [C, HALF], F32, kind="ExternalInput")
    out_d = nc.dram_tensor("out", [C, HALF], F32, kind="ExternalOutput")

    with tile.TileContext(nc) as tc:
        with (
            tc.tile_pool(name="big", bufs=1) as big,
            tc.tile_pool(name="small", bufs=1) as small,
            tc.tile_pool(name="sqs", bufs=2) as sqs,
            tc.tile_pool(name="upool", bufs=6) as upool,
            tc.tile_pool(name="vpool", bufs=6) as vpool,
            tc.tile_pool(name="wpool", bufs=3) as wpool,
            tc.tile_pool(name="opool", bufs=3) as opool,
            tc.tile_pool(name="ps_sc", bufs=2, space=bass.MemorySpace.PSUM) as ps_sc,
            tc.tile_pool(name="ps_rec", bufs=1, space=bass.MemorySpace.PSUM) as ps_rec,
            tc.tile_pool(name="ps_sum", bufs=1, space=bass.MemorySpace.PSUM) as ps_sum,
        ):
            # ---- persistent SBUF tensors ----
            xb_sb = big.tile([C, L], BF, tag="xb_sb")      # mm1 stationary (c,l)
            xt_sb = big.tile([C, L], BF, tag="xt_sb")      # l-major tiles (l,c)
            kn = big.tile([C, L], BF, tag="kn")            # normalized k, l-major
            xyh_sb = big.tile([C, YW], BF, tag="xyh_sb")
            y1 = big.tile([C, YW], BF, tag="y1")
            y_t = big.tile([C, HALF], BF, tag="y_t")
            mrep_sb = big.tile([C, HALF], F32, tag="mrep_sb")
            w_t = big.tile([C, HALF], F32, tag="w_t")      # (1-m)/9
            xm = big.tile([C, HALF], F32, tag="xm")        # x*m
            ones_t = small.tile([C, C], BF, tag="ones_t")
            norm2 = small.tile([C, NLT], F32, tag="norm2")
            rs_a = small.tile([C, NLT], F32, tag="rs_a")
            rs_b = small.tile([C, NLT], F32, tag="rs_b")
            nt_a = small.tile([C, NLT], F32, tag="nt_a")
            rs184 = small.tile([C, NLT], F32, tag="rs184")
            shift_c = small.tile([C, 1], F32, tag="shift_c")
            warm2 = small.tile([C, 1], F32, tag="warm2")

            # ---- input DMAs: only SP (sync) and ACT (scalar) have HW DMA
            # queues (~50GB/s each); order chunks by when they are needed.
            # scalar: xt q0-q2, xb q2, xt q3, xb q3; sync: xyh, xb q0-q1, mrep
            Q = L // 4
            nc.scalar.dma_start(xt_sb[:, 0:Q], xt_d[:, 0:Q])
            nc.sync.dma_start(xyh_sb[:], xyh_d[:])
            nc.sync.dma_start(xb_sb[:, 0:Q], xb_d[:, 0:Q])
            nc.scalar.dma_start(xt_sb[:, Q:2 * Q], xt_d[:, Q:2 * Q])
            nc.sync.dma_start(xb_sb[:, Q:2 * Q], xb_d[:, Q:2 * Q])
            nc.scalar.dma_start(xt_sb[:, 2 * Q:3 * Q], xt_d[:, 2 * Q:3 * Q])
            nc.scalar.dma_start(xb_sb[:, 2 * Q:3 * Q], xb_d[:, 2 * Q:3 * Q])
            nc.scalar.dma_start(xt_sb[:, 3 * Q:L], xt_d[:, 3 * Q:L])
            nc.scalar.dma_start(xb_sb[:, 3 * Q:L], xb_d[:, 3 * Q:L])
            nc.sync.dma_start(mrep_sb[:], mrep_d[:])

            # ---- tiny prologue constants ----
            nc.vector.memset(ones_t[:], 1.0)
            nc.vector.memset(shift_c[:], -SHIFT)
            # pay the exp table-set load (~2.7us) during the DMA window
            nc.scalar.activation(warm2[:], shift_c[:], AF.Exp)

            # norm2[l] = sum_c xt[l, c]^2: ACT (Square+accum) for the first
            # half, during the DMA head while ACT is idle; DVE
            # tensor_tensor_reduce for the second half
            def sq_chunk_act(l0, l1):
                for lt in range(l0, l1):
                    scr = sqs.tile([C, C], BF, tag="sq_scratch")
                    nc.scalar.activation(
                        scr[:], xt_sb[:, lt * C:(lt + 1) * C], AF.Square,
                        accum_out=norm2[:, lt:lt + 1])

            def sq_chunk_dve(l0, l1):
                for lt in range(l0, l1):
                    scr = sqs.tile([C, C], BF, tag="sq_scratch")
                    nc.vector.scalar_tensor_tensor(
                        scr[:], xt_sb[:, lt * C:(lt + 1) * C], 1.0,
                        xt_sb[:, lt * C:(lt + 1) * C],
                        op0=ALU.mult, op1=ALU.mult,
                        accum_out=norm2[:, lt:lt + 1])

            # rsqrt via bit-trick seed + 2 Newton iterations (DVE, f32);
            # seed lands in rs_a, each iteration writes dst then swaps, so an
            # even iteration count ends back in rs_a
            rs_fin = rs_a

            def newton_chunk(l0, l1):
                cl = slice(l0, l1)
                nc.vector.tensor_scalar(nt_a[:, cl].bitcast(I32),
                                        norm2[:, cl].bitcast(I32), 1, None,
                                        op0=ALU.logical_shift_right)
                nc.vector.tensor_scalar(rs_a[:, cl].bitcast(I32),
                                        nt_a[:, cl].bitcast(I32),
                                        -1, 0x5f3759df,
                                        op0=ALU.mult, op1=ALU.add)
                src, dst = rs_a, rs_b
                for _ in range(2):
                    nc.vector.tensor_mul(nt_a[:, cl], src[:, cl], src[:, cl])
                    nc.vector.tensor_mul(nt_a[:, cl], nt_a[:, cl], norm2[:, cl])
                    nc.vector.tensor_scalar(nt_a[:, cl], nt_a[:, cl], -0.5, 1.5,
                                            op0=ALU.mult, op1=ALU.add)
                    nc.vector.tensor_mul(dst[:, cl], src[:, cl], nt_a[:, cl])
                    src, dst = dst, src
                nc.vector.tensor_scalar_mul(rs184[:, cl], rs_fin[:, cl],
                                            LN2_INV_128)

            def kn_chunk(l0, l1, eng):
                for lt in range(l0, l1):
                    eng.tensor_scalar_mul(
                        kn[:, lt * C:(lt + 1) * C], xt_sb[:, lt * C:(lt + 1) * C],
                        rs_fin[:, lt:lt + 1])

            # y = 3x3 box filter (row filter on xyh -> y1, then col filter)
            xv = xyh_sb[:].rearrange("p (r j) -> p r j", j=64)
            yv = y1[:].rearrange("p (r j) -> p r j", j=64)

            # --- critical-path-ordered prologue emission ---
            sq_chunk_act(0, 8)
            # mini y-chain: rows 0..9 only, to unblock the first mm1 ~3us
            # earlier (the rest of part A covers rows 10..18 disjointly)
            nc.vector.tensor_add(y1[:, 1:639], xyh_sb[:, 0:638],
                                 xyh_sb[:, 1:639])
            nc.vector.tensor_add(y1[:, 1:639], y1[:, 1:639],
                                 xyh_sb[:, 2:640])
            nc.vector.tensor_add(yv[:, 0:10, 0:1], xv[:, 0:10, 0:1],
                                 xv[:, 0:10, 1:2])
            nc.vector.tensor_add(yv[:, 0:10, 63:64], xv[:, 0:10, 62:63],
                                 xv[:, 0:10, 63:64])
            nc.vector.tensor_add(y_t[:, 0:512], y1[:, 0:512],
                                 y1[:, 64:64 + 512])
            nc.vector.tensor_add(y_t[:, 0:512], y_t[:, 0:512],
                                 y1[:, 128:128 + 512])
            newton_chunk(0, 8)
            # rest of part A: rows 10..18 (interior flats [641:1216))
            nc.vector.tensor_add(y1[:, 641:1216], xyh_sb[:, 640:1215],
                                 xyh_sb[:, 641:1216])
            nc.vector.tensor_add(y1[:, 641:1216], y1[:, 641:1216],
                                 xyh_sb[:, 642:1217])
            nc.vector.tensor_add(yv[:, 10:19, 0:1], xv[:, 10:19, 0:1],
                                 xv[:, 10:19, 1:2])
            nc.vector.tensor_add(yv[:, 10:19, 63:64], xv[:, 10:19, 62:63],
                                 xv[:, 10:19, 63:64])
            nc.vector.tensor_add(y_t[:, 512:BLK], y1[:, 512:BLK],
                                 y1[:, 512 + 64:64 + BLK])
            nc.vector.tensor_add(y_t[:, 512:BLK], y_t[:, 512:BLK],
                                 y1[:, 512 + 128:128 + BLK])
            kn_chunk(0, 8, nc.vector)
            # (later l-range prep chains are interleaved into block 0 below,
            # paced by when their xt/xb DMA quarters land)

            def emit_exp(u, sc, lt, eng):
                if eng == "A":
                    nc.scalar.activation(u[:], sc[:], AF.Exp,
                                         bias=shift_c[:],
                                         scale=rs_fin[:, lt:lt + 1])
                else:
                    nc.vector.tensor_scalar(u[:].bitcast(I16), sc[:],
                                            rs184[:, lt:lt + 1], SCHR_OFF,
                                            op0=ALU.mult, op1=ALU.add)

            # ---- main loop ----
            # tree structure per block: groups of 8 for lt 0..23, group of 4
            # for lt 24..27, direct ones-mm for lt 28..31. 8 ones-emissions.
            N_ONES = 8
            for blk in range(HALF // BLK):
                rec = ps_rec.tile([C, BLK], F32, tag="rec")
                sums = ps_sum.tile([C, BLK], F32, tag="sums")
                ones_idx = 0
                w_queue = []       # pending (w_tile) for lagged ones-mm
                tree = {}          # partial sums of current group

                def emit_ones(w):
                    nonlocal ones_idx
                    for h2 in range(BLK // 512):
                        nc.tensor.matmul(
                            sums[:, h2 * 512:(h2 + 1) * 512],
                            ones_t[:],
                            w[:, h2 * 512:(h2 + 1) * 512],
                            start=(ones_idx == 0), stop=(ones_idx == N_ONES - 1),
                        )
                    ones_idx += 1

                for lt in range(NLT):
                    # interleave remaining prologue chains off the head,
                    # paced by DMA-quarter landing times (block 0 only)
                    if blk == 0:
                        if lt == 3:
                            sq_chunk_dve(8, 16)
                        elif lt == 5:
                            newton_chunk(8, 16)
                        elif lt == 6:
                            kn_chunk(8, 16, nc.vector)
                            # part B of y1: rows 19..33 -> y_t block 1 (big
                            # adds on GPSIMD, which has slack; edges on DVE)
                            nc.gpsimd.tensor_add(y1[:, 1216:YW - 1],
                                                 xyh_sb[:, 1215:YW - 2],
                                                 xyh_sb[:, 1216:YW - 1])
                            nc.gpsimd.tensor_add(y1[:, 1216:YW - 1],
                                                 y1[:, 1216:YW - 1],
                                                 xyh_sb[:, 1217:YW])
                            nc.vector.tensor_add(yv[:, 19:34, 0:1],
                                                 xv[:, 19:34, 0:1],
                                                 xv[:, 19:34, 1:2])
                            nc.vector.tensor_add(yv[:, 19:34, 63:64],
                                                 xv[:, 19:34, 62:63],
                                                 xv[:, 19:34, 63:64])
                            nc.gpsimd.tensor_add(y_t[:, BLK:HALF],
                                                 y1[:, BLK:BLK + BLK],
                                                 y1[:, BLK + 64:BLK + 64 + BLK])
                            nc.gpsimd.tensor_add(y_t[:, BLK:HALF],
                                                 y_t[:, BLK:HALF],
                                                 y1[:, BLK + 128:BLK + 128 + BLK])
                        elif lt == 8:
                            sq_chunk_dve(16, 24)
                        elif lt == 10:
                            newton_chunk(16, 24)
                        elif lt == 12:
                            kn_chunk(16, 24, nc.vector)
                        elif lt == 16:
                            sq_chunk_dve(24, 32)
                        elif lt == 18:
                            newton_chunk(24, 32)
                        elif lt == 20:
                            kn_chunk(24, 32, nc.vector)
                        elif lt == 22:
                            # x*m from the xyh center rows (bf16 x) - no
                            # separate f32 x DMA needed
                            nc.vector.tensor_mul(xm[:], xyh_sb[:, 64:64 + HALF],
                                                 mrep_sb[:])
                        elif lt == 24:
                            nc.gpsimd.tensor_scalar(w_t[:], mrep_sb[:],
                                                    -1.0 / 9.0, 1.0 / 9.0,
                                                    op0=ALU.mult, op1=ALU.add)
                    sc = ps_sc.tile([C, BLK], F32, tag="sc")
                    for h2 in range(BLK // 512):
                        cs = blk * BLK + h2 * 512
                        nc.tensor.matmul(
                            sc[:, h2 * 512:(h2 + 1) * 512],
                            xb_sb[:, lt * C:(lt + 1) * C],
                            y_t[:, cs:cs + 512],
                            start=True, stop=True,
                        )
                    u = upool.tile([C, BLK], BF, tag="u")
                    emit_exp(u, sc, lt, "D" if lt in DVE_TILES else "A")
                    for h2 in range(BLK // 512):
                        nc.tensor.matmul(
                            rec[:, h2 * 512:(h2 + 1) * 512],
                            kn[:, lt * C:(lt + 1) * C],
                            u[:, h2 * 512:(h2 + 1) * 512],
                            start=(lt == 0), stop=(lt == NLT - 1),
                        )
                    # column-sum handling
                    if lt >= NLT - 4:
                        # direct ones-mm on the last 4 tiles (short tail)
                        emit_ones(u)
                        if lt == NLT - 4:
                            for w in w_queue:
                                emit_ones(w)
                            w_queue = []
                    else:
                        gsz = 8 if lt < 24 else 4
                        pos = lt % gsz
                        if pos % 2 == 0:
                            tree["u"] = u
                        else:
                            v = vpool.tile([C, BLK], BF, tag="v")
                            # pair adds u0+u1/u4+u5 go to GPSIMD (slow per-op
                            # but idle, and these have latency headroom); the
                            # rest of the chain stays on DVE
                            eng = nc.gpsimd if pos in (1, 5) else nc.vector
                            eng.tensor_add(v[:], tree.pop("u")[:], u[:])
                            if pos == 1:
                                tree["v1"] = v
                            elif pos == 3:
                                v2 = vpool.tile([C, BLK], BF, tag="v2")
                                nc.vector.tensor_add(v2[:], tree.pop("v1")[:],
                                                     v[:])
                                if gsz == 4:
                                    w_queue.append(v2)
                                else:
                                    tree["s12"] = v2
                            elif pos == 5:
                                tree["v3"] = v
                            else:  # pos == 7
                                v2 = vpool.tile([C, BLK], BF, tag="v2")
                                nc.vector.tensor_add(v2[:], tree.pop("v3")[:],
                                                     v[:])
                                w = wpool.tile([C, BLK], BF, tag="w")
                                nc.vector.tensor_add(w[:], tree.pop("s12")[:],
                                                     v2[:])
                                w_queue.append(w)
                            # lag the ones-mm ~1 group behind the DVE chain
                            if len(w_queue) > 1:
                                emit_ones(w_queue.pop(0))
                # epilogue: out = rec/sums * (1-m)/9 + x*m  (per-512 pipelined)
                for h2 in range(BLK // 512):
                    cs = blk * BLK + h2 * 512
                    sl = slice(h2 * 512, (h2 + 1) * 512)
                    R = opool.tile([C, 512], F32, tag="R")
                    nc.vector.reciprocal_approx_fast(R[:], sums[:, sl])
                    Rm = opool.tile([C, 512], F32, tag="Rm")
                    nc.vector.tensor_mul(Rm[:], R[:], w_t[:, cs:cs + 512])
                    ob = opool.tile([C, 512], F32, tag="ob")
                    nc.vector.tensor_mul(ob[:], rec[:, sl], Rm[:])
                    nc.vector.tensor_add(ob[:], ob[:], xm[:, cs:cs + 512])
                    nc.sync.dma_start(out_d[:, cs:cs + 512], ob[:])

    nc.finalize()
    return nc


def _get_program():
    if "nc" not in _CACHE:
        _CACHE["nc"] = _build_program()
    return _CACHE["nc"]


def _make_in_maps(fg, mk):
    in_maps = []
    for core in range(8):
        b, h = core // 2, core % 2
        x = np.ascontiguousarray(fg[b].reshape(C, L))
        xb = x.astype(NPBF16)
        # pre-tiled transpose: xt[p, t*128+c] = x[c, t*128+p]
        xt = np.ascontiguousarray(
            x.reshape(C, L // C, C).transpose(2, 1, 0).reshape(C, L)).astype(NPBF16)
        xi = fg[b].reshape(C, 64, 64)
        rows = np.zeros((C, 34, 64), np.float32)
        r0 = 32 * h - 1
        lo, hi = max(0, r0), min(64, r0 + 34)
        rows[:, lo - r0:hi - r0, :] = xi[:, lo:hi, :]
        xyh = np.ascontiguousarray(rows.reshape(C, YW)).astype(NPBF16)
        mrow = mk[b].reshape(1, L)[:, h * HALF:(h + 1) * HALF]
        mrep = np.ascontiguousarray(np.broadcast_to(mrow, (C, HALF)))
        in_maps.append({"xb": xb, "xt": xt, "xyh": xyh, "mrep": mrep})
    return in_maps


def kernel(foreground, mask):
    fg = np.ascontiguousarray(np.asarray(foreground, dtype=np.float32))
    mk = np.ascontiguousarray(np.asarray(mask, dtype=np.float32))
    nc = _get_program()
    in_maps = _make_in_maps(fg, mk)

    from concourse.bass_utils import run_bass_kernel_spmd
    res = run_bass_kernel_spmd(nc, in_maps, core_ids=list(range(8)))

    out = np.empty((4, C, L), np.float32)
    for core in range(8):
        b, h = core // 2, core % 2
        out[b][:, h * HALF:(h + 1) * HALF] = res.results[core]["out"]
    return out.reshape(4, C, 64, 64)


# revision 28
# speedup vs baseline: 1.0089x; 1.0089x over previous
"""Trainium2 Bass kernel for CAttention (contextual attention).

Math (per batch element, derived from the reference):
    x:    (c=128, h=64, w=64), flat (128, 4096); m: (1, 4096)
    k    = normalize_rows(x.reshape(c, hw).T + eps)          # (4096, 128)
    y    = 3x3 zero-padded box filter of x                   # (128, 4096)
    S    = k @ y                                             # (4096 l, 4096 ij)
    att  = softmax over l (per column); u = exp(S - 20) (S bounded, col max
           >= ~11, so a constant shift suffices; att = u / colsum(u))
    rec  = k.T @ att                                         # (128, 4096)
    out  = rec * (1-m)/9 + x*m

Sharding: pure data parallel over batch (4) x output-column halves (2) = 8
cores, zero cross-core communication.

Engine split per core (all matmul operands bf16; 1 cyc/row, same rate as
f32r but half the SBUF/DMA, and 2x faster DVE adds):
  PE   : mm1 S-tiles, mm2 rec accumulation, ones-matmuls for column sums
         of tree-reduced u tiles.                              ~61us
  ACT  : 22 exp tiles per block (exact Exp, bf16 out, per-partition
         scale=rsqrt(norm), bias=-20) + first half of the norm squares
         (AF.Square with accum) during the DMA head.           ~64us
  DVE  : 10 exp tiles per block via the Schraudolph bit-trick (bf16 bits
         of exp(t) = int16(t*128/ln2 + 16256 - C), a single tensor_scalar
         f32->int16), y box filter (bf16 2x adds), second half of squares
         (tensor_tensor_reduce), rsqrt Newton, 4/7 of the column-sum tree,
         epilogue.                                             ~63us
  GPSIMD (cannot read PSUM -> no exp, no scalar_tensor_tensor): all of
         kn = xt*rs, 3/7 of the column-sum tree adds, blend prep. ~55us
Validated numerics: rel err ~2e-4 vs fp64 reference (gate 2e-2).
"""

import numpy as np
import ml_dtypes

NPBF16 = ml_dtypes.bfloat16

EPS = 1e-7
SHIFT = 20.0
LN2_INV_128 = 128.0 / float(np.log(2.0))   # 184.6617
SCHR_C = 5.5
SCHR_OFF = 16256.0 - SHIFT * LN2_INV_128 - SCHR_C
C = 128          # channels
L = 4096         # spatial locations (l axis)
HALF = 2048      # output columns per core
BLK = 1024       # ij block (psum-bank sized: 2 banks)
NLT = 32         # l tiles of 128
YW = 2176        # xyh width: 34 padded image rows x 64

# exp tiles per block handled by DVE (Schraudolph); the rest go to ACT.
# block 0 is paced by input DMA, so ACT can carry more exps there; block 1
# shifts more onto DVE (which is lighter once the prologue chains are done)
DVE_TILES = [{4, 9, 14, 19, 24, 29},
             {2, 5, 8, 11, 14, 17, 20, 23, 26, 29}]

_CACHE = {}


def _build_program():
    import concourse.bass as bass
    import concourse.bacc as bacc
    import concourse.tile as tile
    import concourse.mybir as mybir

    F32 = mybir.dt.float32
    BF = mybir.dt.bfloat16
    I16 = mybir.dt.int16
    I32 = mybir.dt.int32
    AF = mybir.ActivationFunctionType
    ALU = mybir.AluOpType

    nc = bacc.Bacc("TRN2", target_bir_lowering=False, num_swdge_queues=4)

    xb_d = nc.dram_tensor("xb", [C, L], BF, kind="ExternalInput")
    # xt pre-tiled on host to SBUF layout: xt[p, t*128+c] = x[c, t*128+p]
    xt_d = nc.dram_tensor("xt", [C, L], BF, kind="ExternalInput")
    xyh_d = nc.dram_tensor("xyh", [C, YW], BF, kind="ExternalInput")
    mrep_d = nc.dram_tensor("mrep", [C, HALF], F32, kind="ExternalInput")
    out_d = nc.dram_tensor("out", [C, HALF], F32, kind="ExternalOutput")

    with tile.TileContext(nc) as tc:
        with (
            tc.tile_pool(name="big", bufs=1) as big,
            tc.tile_pool(name="small", bufs=1) as small,
            tc.tile_pool(name="sqs", bufs=2) as sqs,
            tc.tile_pool(name="upool", bufs=6) as upool,
            tc.tile_pool(name="vpool", bufs=6) as vpool,
            tc.tile_pool(name="wpool", bufs=3) as wpool,
            tc.tile_pool(name="opool", bufs=3) as opool,
            tc.tile_pool(name="ps_sc", bufs=2, space=bass.MemorySpace.PSUM) as ps_sc,
            tc.tile_pool(name="ps_rec", bufs=1, space=bass.MemorySpace.PSUM) as ps_rec,
            tc.tile_pool(name="ps_sum", bufs=1, space=bass.MemorySpace.PSUM) as ps_sum,
        ):
            # ---- persistent SBUF tensors ----
            xb_sb = big.tile([C, L], BF, tag="xb_sb")      # mm1 stationary (c,l)
            xt_sb = big.tile([C, L], BF, tag="xt_sb")      # l-major tiles (l,c)
            kn = big.tile([C, L], BF, tag="kn")            # normalized k, l-major
            xyh_sb = big.tile([C, YW], BF, tag="xyh_sb")
            y1 = big.tile([C, YW], BF, tag="y1")
            y_t = big.tile([C, HALF], BF, tag="y_t")
            mrep_sb = big.tile([C, HALF], F32, tag="mrep_sb")
            w_t = big.tile([C, HALF], F32, tag="w_t")      # (1-m)/9
            xm = big.tile([C, HALF], F32, tag="xm")        # x*m
            ones_t = small.tile([C, C], BF, tag="ones_t")
            norm2 = small.tile([C, NLT], F32, tag="norm2")
            rs_a = small.tile([C, NLT], F32, tag="rs_a")
            rs_b = small.tile([C, NLT], F32, tag="rs_b")
            nt_a = small.tile([C, NLT], F32, tag="nt_a")
            rs184 = small.tile([C, NLT], F32, tag="rs184")
            shift_c = small.tile([C, 1], F32, tag="shift_c")
            warm2 = small.tile([C, 1], F32, tag="warm2")

            # ---- input DMAs: only SP (sync) and ACT (scalar) have HW DMA
            # queues (~50GB/s each); order chunks by when they are needed.
            # scalar: xt q0-q2, xb q2, xt q3, xb q3; sync: xyh, xb q0-q1, mrep
            Q = L // 4
            nc.scalar.dma_start(xt_sb[:, 0:Q], xt_d[:, 0:Q])
            nc.sync.dma_start(xyh_sb[:], xyh_d[:])
            nc.sync.dma_start(xb_sb[:, 0:Q], xb_d[:, 0:Q])
            nc.scalar.dma_start(xt_sb[:, Q:2 * Q], xt_d[:, Q:2 * Q])
            nc.sync.dma_start(xb_sb[:, Q:2 * Q], xb_d[:, Q:2 * Q])
            nc.scalar.dma_start(xt_sb[:, 2 * Q:3 * Q], xt_d[:, 2 * Q:3 * Q])
            nc.scalar.dma_start(xb_sb[:, 2 * Q:3 * Q], xb_d[:, 2 * Q:3 * Q])
            nc.scalar.dma_start(xt_sb[:, 3 * Q:L], xt_d[:, 3 * Q:L])
            nc.scalar.dma_start(xb_sb[:, 3 * Q:L], xb_d[:, 3 * Q:L])
            nc.sync.dma_start(mrep_sb[:], mrep_d[:])

            # ---- tiny prologue constants ----
            nc.vector.memset(ones_t[:], 1.0)
            nc.vector.memset(shift_c[:], -SHIFT)
            # pay the exp table-set load (~2.7us) during the DMA window
            nc.scalar.activation(warm2[:], shift_c[:], AF.Exp)

            # norm2[l] = sum_c xt[l, c]^2: ACT (Square+accum) for the first
            # half, during the DMA head while ACT is idle; DVE
            # tensor_tensor_reduce for the second half
            def sq_chunk_act(l0, l1):
                for lt in range(l0, l1):
                    scr = sqs.tile([C, C], BF, tag="sq_scratch")
                    nc.scalar.activation(
                        scr[:], xt_sb[:, lt * C:(lt + 1) * C], AF.Square,
                        accum_out=norm2[:, lt:lt + 1])

            def sq_chunk_dve(l0, l1):
                for lt in range(l0, l1):
                    scr = sqs.tile([C, C], BF, tag="sq_scratch")
                    nc.vector.scalar_tensor_tensor(
                        scr[:], xt_sb[:, lt * C:(lt + 1) * C], 1.0,
                        xt_sb[:, lt * C:(lt + 1) * C],
                        op0=ALU.mult, op1=ALU.mult,
                        accum_out=norm2[:, lt:lt + 1])

            # rsqrt via bit-trick seed + 2 Newton iterations (DVE, f32);
            # seed lands in rs_a, each iteration writes dst then swaps, so an
            # even iteration count ends back in rs_a
            rs_fin = rs_a

            def newton_chunk(l0, l1):
                cl = slice(l0, l1)
                nc.vector.tensor_scalar(nt_a[:, cl].bitcast(I32),
                                        norm2[:, cl].bitcast(I32), 1, None,
                                        op0=ALU.logical_shift_right)
                nc.vector.tensor_scalar(rs_a[:, cl].bitcast(I32),
                                        nt_a[:, cl].bitcast(I32),
                                        -1, 0x5f3759df,
                                        op0=ALU.mult, op1=ALU.add)
                src, dst = rs_a, rs_b
                for _ in range(2):
                    nc.vector.tensor_mul(nt_a[:, cl], src[:, cl], src[:, cl])
                    nc.vector.tensor_mul(nt_a[:, cl], nt_a[:, cl], norm2[:, cl])
                    nc.vector.tensor_scalar(nt_a[:, cl], nt_a[:, cl], -0.5, 1.5,
                                            op0=ALU.mult, op1=ALU.add)
                    nc.vector.tensor_mul(dst[:, cl], src[:, cl], nt_a[:, cl])
                    src, dst = dst, src
                nc.vector.tensor_scalar_mul(rs184[:, cl], rs_fin[:, cl],
                                            LN2_INV_128)

            def kn_chunk(l0, l1, eng):
                for lt in range(l0, l1):
                    eng.tensor_scalar_mul(
                        kn[:, lt * C:(lt + 1) * C], xt_sb[:, lt * C:(lt + 1) * C],
                        rs_fin[:, lt:lt + 1])

            # y = 3x3 box filter (row filter on xyh -> y1, then col filter)
            xv = xyh_sb[:].rearrange("p (r j) -> p r j", j=64)
            yv = y1[:].rearrange("p (r j) -> p r j", j=64)

            # --- critical-path-ordered prologue emission ---
            sq_chunk_act(0, 8)
            # mini y-chain: rows 0..9 only, to unblock the first mm1 ~3us
            # earlier (the rest of part A covers rows 10..18 disjointly)
            nc.vector.tensor_add(y1[:, 1:639], xyh_sb[:, 0:638],
                                 xyh_sb[:, 1:639])
            nc.vector.tensor_add(y1[:, 1:639], y1[:, 1:639],
                                 xyh_sb[:, 2:640])
            nc.vector.tensor_add(yv[:, 0:10, 0:1], xv[:, 0:10, 0:1],
                                 xv[:, 0:10, 1:2])
            nc.vector.tensor_add(yv[:, 0:10, 63:64], xv[:, 0:10, 62:63],
                                 xv[:, 0:10, 63:64])
            nc.vector.tensor_add(y_t[:, 0:512], y1[:, 0:512],
                                 y1[:, 64:64 + 512])
            nc.vector.tensor_add(y_t[:, 0:512], y_t[:, 0:512],
                                 y1[:, 128:128 + 512])
            newton_chunk(0, 8)
            # rest of part A: rows 10..18 (interior flats [641:1216))
            nc.vector.tensor_add(y1[:, 641:1216], xyh_sb[:, 640:1215],
                                 xyh_sb[:, 641:1216])
            nc.vector.tensor_add(y1[:, 641:1216], y1[:, 641:1216],
                                 xyh_sb[:, 642:1217])
            nc.vector.tensor_add(yv[:, 10:19, 0:1], xv[:, 10:19, 0:1],
                                 xv[:, 10:19, 1:2])
            nc.vector.tensor_add(yv[:, 10:19, 63:64], xv[:, 10:19, 62:63],
                                 xv[:, 10:19, 63:64])
            nc.vector.tensor_add(y_t[:, 512:BLK], y1[:, 512:BLK],
                                 y1[:, 512 + 64:64 + BLK])
            nc.vector.tensor_add(y_t[:, 512:BLK], y_t[:, 512:BLK],
                                 y1[:, 512 + 128:128 + BLK])
            kn_chunk(0, 8, nc.vector)
            # (later l-range prep chains are interleaved into block 0 below,
            # paced by when their xt/xb DMA quarters land)

            def emit_exp(u, sc, lt, eng):
                if eng == "A":
                    nc.scalar.activation(u[:], sc[:], AF.Exp,
                                         bias=shift_c[:],
                                         scale=rs_fin[:, lt:lt + 1])
                else:
                    nc.vector.tensor_scalar(u[:].bitcast(I16), sc[:],
                                            rs184[:, lt:lt + 1], SCHR_OFF,
                                            op0=ALU.mult, op1=ALU.add)

            # ---- main loop ----
            # tree structure per block: groups of 8 for lt 0..23, group of 4
            # for lt 24..27, direct ones-mm for lt 28..31. 8 ones-emissions.
            N_ONES = 8
            for blk in range(HALF // BLK):
                rec = ps_rec.tile([C, BLK], F32, tag="rec")
                sums = ps_sum.tile([C, BLK], F32, tag="sums")
                ones_idx = 0
                w_queue = []       # pending (w_tile) for lagged ones-mm
                tree = {}          # partial sums of current group

                def emit_ones(w):
                    nonlocal ones_idx
                    for h2 in range(BLK // 512):
                        nc.tensor.matmul(
                            sums[:, h2 * 512:(h2 + 1) * 512],
                            ones_t[:],
                            w[:, h2 * 512:(h2 + 1) * 512],
                            start=(ones_idx == 0), stop=(ones_idx == N_ONES - 1),
                        )
                    ones_idx += 1

                for lt in range(NLT):
                    # interleave remaining prologue chains off the head,
                    # paced by DMA-quarter landing times (block 0 only)
                    if blk == 0:
                        if lt == 3:
                            sq_chunk_dve(8, 16)
                        elif lt == 5:
                            newton_chunk(8, 16)
                        elif lt == 6:
                            kn_chunk(8, 16, nc.vector)
                            # part B of y1: rows 19..33 -> y_t block 1
                            # (keep all y1/y_t writers on DVE: cross-engine
                            # writers of one tile serialize on semaphores)
                            nc.vector.tensor_add(y1[:, 1216:YW - 1],
                                                 xyh_sb[:, 1215:YW - 2],
                                                 xyh_sb[:, 1216:YW - 1])
                            nc.vector.tensor_add(y1[:, 1216:YW - 1],
                                                 y1[:, 1216:YW - 1],
                                                 xyh_sb[:, 1217:YW])
                            nc.vector.tensor_add(yv[:, 19:34, 0:1],
                                                 xv[:, 19:34, 0:1],
                                                 xv[:, 19:34, 1:2])
                            nc.vector.tensor_add(yv[:, 19:34, 63:64],
                                                 xv[:, 19:34, 62:63],
                                                 xv[:, 19:34, 63:64])
                            nc.vector.tensor_add(y_t[:, BLK:HALF],
                                                 y1[:, BLK:BLK + BLK],
                                                 y1[:, BLK + 64:BLK + 64 + BLK])
                            nc.vector.tensor_add(y_t[:, BLK:HALF],
                                                 y_t[:, BLK:HALF],
                                                 y1[:, BLK + 128:BLK + 128 + BLK])
                        elif lt == 8:
                            sq_chunk_dve(16, 24)
                        elif lt == 10:
                            newton_chunk(16, 24)
                        elif lt == 12:
                            kn_chunk(16, 24, nc.vector)
                        elif lt == 16:
                            sq_chunk_dve(24, 32)
                        elif lt == 18:
                            newton_chunk(24, 32)
                        elif lt == 20:
                            kn_chunk(24, 32, nc.vector)
                        elif lt == 22:
                            # x*m from the xyh center rows (bf16 x) - no
                            # separate f32 x DMA needed. tile_wait_until stops
                            # the scheduler hoisting these ahead of readier
                            # work (mrep lands late; head-of-line blocking)
                            with tc.tile_wait_until(0.030):
                                nc.vector.tensor_mul(xm[:],
                                                     xyh_sb[:, 64:64 + HALF],
                                                     mrep_sb[:])
                        elif lt == 24:
                            with tc.tile_wait_until(0.030):
                                nc.gpsimd.tensor_scalar(w_t[:], mrep_sb[:],
                                                        -1.0 / 9.0, 1.0 / 9.0,
                                                        op0=ALU.mult,
                                                        op1=ALU.add)
                    sc = ps_sc.tile([C, BLK], F32, tag="sc")
                    for h2 in range(BLK // 512):
                        cs = blk * BLK + h2 * 512
                        nc.tensor.matmul(
                            sc[:, h2 * 512:(h2 + 1) * 512],
                            xb_sb[:, lt * C:(lt + 1) * C],
                            y_t[:, cs:cs + 512],
                            start=True, stop=True,
                        )
                    u = upool.tile([C, BLK], BF, tag="u")
                    emit_exp(u, sc, lt, "D" if lt in DVE_TILES[blk] else "A")
                    for h2 in range(BLK // 512):
                        nc.tensor.matmul(
                            rec[:, h2 * 512:(h2 + 1) * 512],
                            kn[:, lt * C:(lt + 1) * C],
                            u[:, h2 * 512:(h2 + 1) * 512],
                            start=(lt == 0), stop=(lt == NLT - 1),
                        )
                    # column-sum handling
                    if lt >= NLT - 4:
                        # direct ones-mm on the last 4 tiles (short tail)
                        emit_ones(u)
                        if lt == NLT - 4:
                            for w in w_queue:
                                emit_ones(w)
                            w_queue = []
                    else:
                        gsz = 8 if lt < 24 else 4
                        pos = lt % gsz
                        if pos % 2 == 0:
                            tree["u"] = u
                        else:
                            v = vpool.tile([C, BLK], BF, tag="v")
                            # pair adds u0+u1/u4+u5 go to GPSIMD (slow per-op
                            # but idle, and these have latency headroom); the
                            # rest of the chain stays on DVE
                            eng = nc.gpsimd if pos in (1, 5) else nc.vector
                            eng.tensor_add(v[:], tree.pop("u")[:], u[:])
                            if pos == 1:
                                tree["v1"] = v
                            elif pos == 3:
                                v2 = vpool.tile([C, BLK], BF, tag="v2")
                                nc.vector.tensor_add(v2[:], tree.pop("v1")[:],
                                                     v[:])
                                if gsz == 4:
                                    w_queue.append(v2)
                                else:
                                    tree["s12"] = v2
                            elif pos == 5:
                                tree["v3"] = v
                            else:  # pos == 7
                                v2 = vpool.tile([C, BLK], BF, tag="v2")
                                nc.vector.tensor_add(v2[:], tree.pop("v3")[:],
                                                     v[:])
                                w = wpool.tile([C, BLK], BF, tag="w")
                                nc.vector.tensor_add(w[:], tree.pop("s12")[:],
                                                     v2[:])
                                w_queue.append(w)
                            # lag the ones-mm ~1 group behind the DVE chain
                            if len(w_queue) > 1:
                                emit_ones(w_queue.pop(0))
                # epilogue: out = rec/sums * (1-m)/9 + x*m  (per-512 pipelined)
                for h2 in range(BLK // 512):
                    cs = blk * BLK + h2 * 512
                    sl = slice(h2 * 512, (h2 + 1) * 512)
                    R = opool.tile([C, 512], F32, tag="R")
                    nc.vector.reciprocal_approx_fast(R[:], sums[:, sl])
                    Rm = opool.tile([C, 512], F32, tag="Rm")
                    nc.vector.tensor_mul(Rm[:], R[:], w_t[:, cs:cs + 512])
                    ob = opool.tile([C, 512], F32, tag="ob")
                    nc.vector.tensor_mul(ob[:], rec[:, sl], Rm[:])
                    nc.vector.tensor_add(ob[:], ob[:], xm[:, cs:cs + 512])
                    nc.sync.dma_start(out_d[:, cs:cs + 512], ob[:])

    nc.finalize()
    return nc


def _get_program():
    if "nc" not in _CACHE:
        _CACHE["nc"] = _build_program()
    return _CACHE["nc"]


def _make_in_maps(fg, mk):
    in_maps = []
    for core in range(8):
        b, h = core // 2, core % 2
        x = np.ascontiguousarray(fg[b].reshape(C, L))
        xb = x.astype(NPBF16)
        # pre-tiled transpose: xt[p, t*128+c] = x[c, t*128+p]
        xt = np.ascontiguousarray(
            x.reshape(C, L // C, C).transpose(2, 1, 0).reshape(C, L)).astype(NPBF16)
        xi = fg[b].reshape(C, 64, 64)
        rows = np.zeros((C, 34, 64), np.float32)
        r0 = 32 * h - 1
        lo, hi = max(0, r0), min(64, r0 + 34)
        rows[:, lo - r0:hi - r0, :] = xi[:, lo:hi, :]
        xyh = np.ascontiguousarray(rows.reshape(C, YW)).astype(NPBF16)
        mrow = mk[b].reshape(1, L)[:, h * HALF:(h + 1) * HALF]
        mrep = np.ascontiguousarray(np.broadcast_to(mrow, (C, HALF)))
        in_maps.append({"xb": xb, "xt": xt, "xyh": xyh, "mrep": mrep})
    return in_maps


def kernel(foreground, mask):
    fg = np.ascontiguousarray(np.asarray(foreground, dtype=np.float32))
    mk = np.ascontiguousarray(np.asarray(mask, dtype=np.float32))
    nc = _get_program()
    in_maps = _make_in_maps(fg, mk)

    from concourse.bass_utils import run_bass_kernel_spmd
    res = run_bass_kernel_spmd(nc, in_maps, core_ids=list(range(8)))

    out = np.empty((4, C, L), np.float32)
    for core in range(8):
        b, h = core // 2, core % 2
        out[b][:, h * HALF:(h + 1) * HALF] = res.results[core]["out"]
    return out.reshape(4, C, 64, 64)
